# revision 12
# baseline (speedup 1.0000x reference)
"""MQA attention (16 Q heads, 1 KV head) on 8 trn2 NeuronCores.

Sharding: data-parallel on batch (2) x tensor-parallel on Q heads (4 per
core). Each core computes K/V for its batch (replicated within the batch
group), attention for its 4 heads, and a row-parallel o_proj partial; the
host sums the 4 partials per batch.

Per-core kernel layout strategy: all matmul contractions on partitions.
  xT [1024, 2048] (host pre-transposed)
  qT = wqT.T @ xT -> [256, 2048] as 2 head-pair tiles [128, 2048]
  kT duplicated to both partition halves -> row-packed score matmuls
     (K=64 per head, 2 heads share the 128 PE rows)
  scoresT [k, q] per (pair, qchunk, kblock) in PSUM [128, 1024] (2 heads)
  exp on ScalarE PSUM->SBUF with per-partition (=per-key) mask bias
  PV: lhsT = [v | ones] [128, 65] -> attn_outT [64, q] + denominator row
  normalize: reciprocal + DMA partition-broadcast + DVE multiply
  o_proj: out[q,
 hidden] partial = attnT.T @ woT, K=256
"""
import sys

sys.path.insert(0, "/opt/trn_rl_repo")

import numpy as np

import concourse.bass as bass
import concourse.bacc as bacc
import concourse.tile as tile
from concourse import mybir
from concourse.bass_utils import run_bass_kernel_spmd
from concourse.tile_rust import add_dep_helper

HIDDEN = 1024
NH = 16
D = 64
B = 2
S = 2048
NCORES = 8
HEADS_PER_CORE = 4
KB = S // 128   # 16 key blocks
QC = S // 512   # 4 query chunks
P = 128

F32 = mybir.dt.float32
F32R = mybir.dt.float32r

_CACHE = {}


def build_kernel(debug_taps=False):
    nc = bacc.Bacc("TRN2", target_bir_lowering=False, debug=False,
                   num_devices=NCORES)

    xT = nc.dram_tensor("xT", [HIDDEN, S], F32R, kind="ExternalInput")
    wqT = nc.dram_tensor("wqT", [HIDDEN, 256], F32R, kind="ExternalInput")
    wkkT = nc.dram_tensor("wkkT", [HIDDEN, 128], F32R, kind="ExternalInput")
    wvT = nc.dram_tensor("wvT", [HIDDEN, D], F32R, kind="ExternalInput")
    woT = nc.dram_tensor("woT", [256, HIDDEN], F32R, kind="ExternalInput")
    bias2d = nc.dram_tensor("bias2d", [P, KB], F32, kind="ExternalInput")
    ones2d = nc.dram_tensor("ones2d", [P, KB], F32R, kind="ExternalInput")
    out = nc.dram_tensor("out", [S, HIDDEN], F32, kind="ExternalOutput")
    # internal DRAM bounce for the per-query 1/denom row broadcast
    bounce = nc.dram_tensor("bounce", [QC, 2, 2, 512], F32)
    if debug_taps:
        qt_dbg = nc.dram_tensor("qt_dbg", [P, 2, S], F32, kind="ExternalOutput")
        kt_dbg = nc.dram_tensor("kt_dbg", [P, S], F32, kind="ExternalOutput")
        va_dbg = nc.dram_tensor("va_dbg", [P, KB, D + 1], F32, kind="ExternalOutput")
        at_dbg = nc.dram_tensor("at_dbg", [P, 2, S], F32, kind="ExternalOutput")
        sc_dbg = nc.dram_tensor("sc_dbg", [P, 1024], F32, kind="ExternalOutput")
        ex_dbg = nc.dram_tensor("ex_dbg", [P, 1024], F32, kind="ExternalOutput")
        aa_dbg = nc.dram_tensor("aa_dbg", [D + 1, 512], F32, kind="ExternalOutput")
        rec_dbg = nc.dram_tensor("rec_dbg", [1, 512], F32, kind="ExternalOutput")
        bc_dbg = nc.dram_tensor("bc_dbg", [D, 512], F32, kind="ExternalOutput")

    with tile.TileContext(nc) as tc:
        with tc.tile_pool(name="persist", bufs=1) as persist:
            xt = persist.tile([P, 8, S], F32R)          # xT tiles, 64KB/part
            qt = persist.tile([P, 2, S], F32R)          # qT head pairs
            kt = persist.tile([P, S], F32R)             # kT dup both halves
            vaug = persist.tile([P, KB, D + 1], F32R)   # [v | ones]
            attnT = persist.tile([P, 2, S], F32R)       # normalized attnT
            wq_sb = persist.tile([P, 8, 256], F32R)
            wkk_sb = persist.tile([P, 8, 128], F32R)
            wv_sb = persist.tile([P, 8, D], F32R)
            wo_sb = persist.tile([P, 2, HIDDEN], F32R)
            bias_sb = persist.tile([P, KB], F32)

            # ---- input DMAs ----
            for kc in range(8):
                nc.sync.dma_start(out=xt[:, kc, :],
                                  in_=xT[kc * P:(kc + 1) * P, :])
                nc.sync.dma_start(out=wq_sb[:, kc, :],
                                  in_=wqT[kc * P:(kc + 1) * P, :])
                nc.sync.dma_start(out=wkk_sb[:, kc, :],
                                  in_=wkkT[kc * P:(kc + 1) * P, :])
                nc.sync.dma_start(out=wv_sb[:, kc, :],
                                  in_=wvT[kc * P:(kc + 1) * P, :])
            for t in range(2):
                nc.sync.dma_start(out=wo_sb[:, t, :],
                                  in_=woT[t * P:(t + 1) * P, :])
            nc.sync.dma_start(out=bias_sb, in_=bias2d[:, :])
            nc.sync.dma_start(out=vaug[:, :, D:D + 1], in_=ones2d[:, :])

            # ---- projections ----
            with tc.tile_pool(name="proj_ps", bufs=2, space="PSUM") as pps:
                # qT: per pair, per q chunk
                for pair in range(2):
                    for j in range(QC):
                        pq = pps.tile([P, 512], F32, tag="pq")
                        for kc in range(8):
                            nc.tensor.matmul(
                                pq,
                                lhsT=wq_sb[:, kc, pair * P:(pair + 1) * P],
                                rhs=xt[:, kc, j * 512:(j + 1) * 512],
                                start=(kc == 0), stop=(kc == 7))
                        nc.vector.tensor_copy(
                            qt[:, pair, j * 512:(j + 1) * 512], pq)
                # kT (duplicated into both partition halves via [wkT|wkT])
                for j in range(QC):
                    pk = pps.tile([P, 512], F32, tag="pk")
                    for kc in range(8):
                        nc.tensor.matmul(
                            pk, lhsT=wkk_sb[:, kc, :],
                            rhs=xt[:, kc, j * 512:(j + 1) * 512],
                            start=(kc == 0), stop=(kc == 7))
                    nc.vector.tensor_copy(kt[:, j * 512:(j + 1) * 512], pk)
                # v natural [S, 64] tiles
                for sc in range(KB):
                    pv = pps.tile([P, D], F32, tag="pv")
                    for kc in range(8):
                        nc.tensor.matmul(
                            pv, lhsT=xt[:, kc, sc * P:(sc + 1) * P],
                            rhs=wv_sb[:, kc, :],
                            start=(kc == 0), stop=(kc == 7))
                    nc.vector.tensor_copy(vaug[:, sc, 0:D], pv)

            # ---- attention ----
            with tc.tile_pool(name="sc_ps", bufs=2, space="PSUM") as scp, \
                 tc.tile_pool(name="att_ps", bufs=2, space="PSUM") as attp, \
                 tc.tile_pool(name="exp_sb", bufs=3) as expp, \
                 tc.tile_pool(name="norm_sb", bufs=2) as normp:
                for j in range(QC):
                    for pair in range(2):
                        attA = attp.tile([D + 1, 512], F32, tag="attA")
                        attB = attp.tile([D + 1, 512], F32, tag="attB")
                        for kb in range(KB):
                            sc = scp.tile([P, 1024], F32, tag="sc")
                            # head even (PE rows 0-63), head odd (rows 64-127)
                            nc.tensor.matmul(
                                sc[:, 0:512],
                                lhsT=kt[0:D, kb * P:(kb + 1) * P],
                                rhs=qt[0:D, pair, j * 512:(j + 1) * 512],
                                start=True, stop=True)
                            nc.tensor.matmul(
                                sc[:, 512:1024],
                                lhsT=kt[D:P, kb * P:(kb + 1) * P],
                                rhs=qt[D:P, pair, j * 512:(j + 1) * 512],
                                start=True, stop=True)
                            ex = expp.tile([P, 1024], F32R, tag="ex")
                            nc.scalar.activation(
                                ex, sc, mybir.ActivationFunctionType.Exp,
                                bias=bias_sb[:, kb:kb + 1], scale=1.0)
                            if debug_taps and j == 0 and pair == 0 and kb == 0:
                                scd = expp.tile([P, 1024], F32, tag="scd")
                                nc.vector.tensor_copy(scd, sc)
                                nc.sync.dma_start(out=sc_dbg.ap(), in_=scd)
                                nc.sync.dma_start(out=ex_dbg.ap(),
                                                  in_=ex.bitcast(F32))
                            nc.tensor.matmul(
                                attA, lhsT=vaug[:, kb, :],
                                rhs=ex[:, 0:512],
                                start=(kb == 0), stop=(kb == KB - 1))
                            nc.tensor.matmul(
                                attB, lhsT=vaug[:, kb, :],
                                rhs=ex[:, 512:1024],
                                start=(kb == 0), stop=(kb == KB - 1))
                        if debug_taps and j == 0 and pair == 0:
                            aad = expp.tile([D + 1, 512], F32, tag="aad")
                            nc.vector.tensor_copy(aad, attA)
                            nc.sync.dma_start(out=aa_dbg.ap(), in_=aad)
                        # normalize: attn_outT[d, q] * (1/denom[q])
                        for h01, attP in ((0, attA), (1, attB)):
                            rec = normp.tile([D + 1, 512], F32, tag="rec")
                            nc.vector.reciprocal(
                                out=rec[D:D + 1, :], in_=attP[D:D + 1, :])
                            bc = normp.tile([D, 1, 512], F32, tag="bc")
                            wdma = nc.sync.dma_start(
                                out=bounce[j, pair, h01, :],
                                in_=rec[D:D + 1, :])
                            rdma = nc.sync.dma_start(
                                out=bc,
                                in_=bounce[j, pair,
                                           h01, :].partition_broadcast(D))
                            add_dep_helper(rdma.ins, wdma.ins,
                                           reason="bounce RAW")
                            if debug_taps and j == 0 and pair == 0 and h01 == 0:
                                nc.sync.dma_start(out=rec_dbg.ap(),
                                                  in_=rec[D:D + 1, :])
                                nc.sync.dma_start(out=bc_dbg.ap(),
                                                  in_=bc[:, 0, :])
                            if h01 == 0:
                                nc.vector.tensor_mul(
                                    attnT[0:D, pair, j * 512:(j + 1) * 512],
                                    attP[0:D, :], bc[:, 0, :])
                            else:
                                nt = normp.tile([D, 512], F32R, tag="nt")
                                nc.vector.tensor_mul(nt, attP[0:D, :],
                                                     bc[:, 0, :])
                                # shift to partitions 64-127 via DMA
                                nc.sync.dma_start(
                                    out=attnT[D:P, pair,
                                              j * 512:(j + 1) * 512],
                                    in_=nt)

            if debug_taps:
                nc.sync.dma_start(out=qt_dbg.ap(), in_=qt.bitcast(F32))
                nc.sync.dma_start(out=kt_dbg.ap(), in_=kt.bitcast(F32))
                nc.sync.dma_start(out=va_dbg.ap(), in_=vaug.bitcast(F32))
                nc.sync.dma_start(out=at_dbg.ap(), in_=attnT.bitcast(F32))

            # ---- o_proj (row-parallel partial) ----
            with tc.tile_pool(name="o_ps", bufs=2, space="PSUM") as ops, \
                 tc.tile_pool(name="o_sb", bufs=3) as osb:
                for sc in range(KB):
                    for n in range(2):
                        po = ops.tile([P, 512], F32, tag="po")
                        for t in range(2):
                            nc.tensor.matmul(
                                po,
                                lhsT=attnT[:, t, sc * P:(sc + 1) * P],
                                rhs=wo_sb[:, t, n * 512:(n + 1) * 512],
                                start=(t == 0), stop=(t == 1))
                        ot = osb.tile([P, 512], F32, tag="ot")
                        nc.vector.tensor_copy(ot, po)
                        nc.sync.dma_start(
                            out=out[sc * P:(sc + 1) * P,
                                    n * 512:(n + 1) * 512],
                            in_=ot)

    nc.finalize()
    return nc


def make_in_maps(hidden_states, attention_mask, wq, wk, wv, wo):
    scale = D ** -0.5
    wq_s = (wq * scale).astype(np.float32)
    in_maps = []
    for c in range(NCORES):
        b = c // 4
        g = c % 4
        h0 = g * HEADS_PER_CORE * D  # first row of this core's q heads
        xTc = np.ascontiguousarray(hidden_states[b].T)
        wqTc = np.ascontiguousarray(wq_s[h0:h0 + 256, :].T)
        wkkTc = np.ascontiguousarray(
            np.concatenate([wk.T, wk.T], axis=1)).astype(np.float32)
        wvTc = np.ascontiguousarray(wv.T)
        woTc = np.ascontiguousarray(wo[:, h0:h0 + 256].T)
        bias = ((1.0 - attention_mask[b]) * -1e30).astype(np.float32)
        bias2d = np.ascontiguousarray(bias.reshape(KB, P).T)
        in_maps.append({
            "xT": xTc.astype(np.float32),
            "wqT": wqTc.astype(np.float32),
            "wkkT": wkkTc,
            "wvT": wvTc.astype(np.float32),
            "woT": woTc.astype(np.float32),
            "bias2d": bias2d,
            "ones2d": np.ones((P, KB), dtype=np.float32),
        })
    return in_maps


def run(inputs, trace=False, trace_cores=None):
    """Compile (cached) and run; returns (full_output, BassKernelResults)."""
    if "nc" not in _CACHE:
        _CACHE["nc"] = build_kernel()
    nc = _CACHE["nc"]
    in_maps = make_in_maps(**inputs)
    res = run_bass_kernel_spmd(
        nc, in_maps, list(range(NCORES)), trace=trace,
        trace_cores=trace_cores)
    parts = [res.results[c]["out"] for c in range(NCORES)]
    full = np.empty((B, S, HIDDEN), dtype=np.float32)
    for b in range(B):
        acc = np.zeros((S, HIDDEN), dtype=np.float64)
        for g in range(4):
            acc += parts[4 * b + g]
        full[b] = acc.astype(np.float32)
    return full, res


def kernel(hidden_states, attention_mask, wq, wk, wv, wo):
    full, _ = run(dict(hidden_states=np.asarray(hidden_states),
                       attention_mask=np.asarray(attention_mask),
                       wq=np.asarray(wq), wk=np.asarray(wk),
                       wv=np.asarray(wv), wo=np.asarray(wo)))
    return full
